# revision 1
# baseline (speedup 1.0000x reference)
"""Trainium2 Bass kernel for Transformer-XL relative attention (nn_Attention).

Sharding: 8 cores = data-parallel over batch (2) x tensor-parallel over heads
(16 -> 4 per core).  Each core computes its 4 heads' attention for its batch,
a partial output projection, then per-half ReduceScatter(add) over its batch
quad; each core LayerNorms its 512-row output shard (tokens [256g, 256g+256)
and [1024+256g, ...+256)).

Device-side structure (per core), restructured for PE density + overlap:
- fp16 matmuls, fp32 PSUM.
- The reference's _rel_shift (shear with cross-row wraparound) is computed
  exactly via a flat DRAM buffer PER (half, head): bd rows written at stride
  L+1 with a leading zero; rows of length L re-read at offset (L - H0) give
  the shifted matrix for that half.  Contiguous f16 DMA both directions
  (HWDGE sync ring for writes, gpsimd for reads).
- scores psum = ac matmul (K=64) + identity-matmul add of shifted bd.
- exp() runs directly from PSUM (scalar engine) -> probs f16 in SBUF;
  PE transposes the probs; context matmul consumes the transposed tiles
  one step behind the transposes to avoid PE stalls.
- mask + softmax denominator ride the context matmul: V+ = [V*mask | mask],
  so psum row 64 is the masked denominator.
- Loop order is half-outer: after each half's 4 heads, the output projection
  chunk + ReduceScatter + LayerNorm for that half are issued, overlapping the
  next half's compute (kills the serial collective tail).
"""

import numpy as np

B, L, D, NH, DH = 2, 2048, 1024, 16, 64
NHL = 4
P = 128
SCALE = 1.0 / np.sqrt(DH)
LN_EPS = 1e-5
N_CORES = 8
PFR = 1025 * 2049  # per-half flat shift buffer (rows 0..1024 incl boundary)

_CACHE = {}


def _build_program():
    import concourse.bacc as bacc
    import concourse.mybir as mybir
    import concourse.tile as tile
    from concourse.masks import make_identity

    F32 = mybir.dt.float32
    F16 = mybir.dt.float16
    AF = mybir.ActivationFunctionType
    AX = mybir.AxisListType
    OP = mybir.AluOpType

    nc = bacc.Bacc("TRN2", target_bir_lowering=False, debug=False,
                   num_devices=N_CORES)

    xT = nc.declare_dram_parameter("xT", [D, L], F16, isOutput=False)
    relT = nc.declare_dram_parameter("relT", [D, L], F16, isOutput=False)
    xres = nc.declare_dram_parameter("xres", [512, D], F32, isOutput=False)
    Wq = nc.declare_dram_parameter("Wq", [D, 256], F16, isOutput=False)
    Wk = nc.declare_dram_parameter("Wk", [D, 256], F16, isOutput=False)
    Wv = nc.declare_dram_parameter("Wv", [D, 256], F16, isOutput=False)
    Wrel = nc.declare_dram_parameter("Wrel", [D, 256], F16, isOutput=False)
    Wout = nc.declare_dram_parameter("Wout", [256, D], F16, isOutput=False)
    rwb = nc.declare_dram_parameter("rwb", [256], F32, isOutput=False)
    rrb = nc.declare_dram_parameter("rrb", [256], F32, isOutput=False)
    mask01 = nc.declare_dram_parameter("mask01", [L], F32, isOutput=False)
    gamma = nc.declare_dram_parameter("gamma", [D], F16, isOutput=False)
    beta = nc.declare_dram_parameter("beta", [D], F16, isOutput=False)
    out = nc.declare_dram_parameter("out", [512, D], F32, isOutput=True)

    from contextlib import ExitStack
    with tile.TileContext(nc) as tc:
        with ExitStack() as _es:
            pers = _es.enter_context(tc.tile_pool(name="persist", bufs=1))
            dram = _es.enter_context(tc.tile_pool(name="dram", bufs=1, space="DRAM"))
            wr_p = _es.enter_context(tc.tile_pool(name="wr", bufs=1))
            slab_p = _es.enter_context(tc.tile_pool(name="slab", bufs=3))
            wt_p = _es.enter_context(tc.tile_pool(name="wt", bufs=3))
            sh_p = _es.enter_context(tc.tile_pool(name="sh", bufs=4))
            p16_p = _es.enter_context(tc.tile_pool(name="p16", bufs=12))
            pt_p = _es.enter_context(tc.tile_pool(name="pt", bufs=2))
            den_p = _es.enter_context(tc.tile_pool(name="den", bufs=1))
            cs_p = _es.enter_context(tc.tile_pool(name="cs", bufs=1))
            bc_p = _es.enter_context(tc.tile_pool(name="bc", bufs=2))
            odd_p = _es.enter_context(tc.tile_pool(name="oddt", bufs=2))
            oc_p = _es.enter_context(tc.tile_pool(name="oc", bufs=3))
            wo_p = _es.enter_context(tc.tile_pool(name="wo", bufs=1))
            ln_p = _es.enter_context(tc.tile_pool(name="ln", bufs=1))
            lng_p = _es.enter_context(tc.tile_pool(name="lng", bufs=1))
            psP = _es.enter_context(tc.tile_pool(name="psP", bufs=4, space="PSUM"))
            psT = _es.enter_context(tc.tile_pool(name="psT", bufs=2, space="PSUM"))
            psC = _es.enter_context(tc.tile_pool(name="psC", bufs=2, space="PSUM"))

            # ---------- persistent setup ----------
            ident = pers.tile([P, P], F16)
            make_identity(nc, ident[:])
            ones_r = pers.tile([1, 64], F16)
            nc.vector.memset(ones_r[:], 1.0)
            nbias = pers.tile([P, 1], F32)
            nc.vector.memset(nbias[:], -4.0)
            m01 = pers.tile([P, 16], F32)
            nc.sync.dma_start(m01[:], mask01.rearrange("(o p) -> p o", p=P))

            rwT = [pers.tile([P, L], F16, name=f"rwT{c}") for c in range(2)]
            rrT = [pers.tile([P, L], F16, name=f"rrT{c}") for c in range(2)]
            kT = [pers.tile([P, L], F16, name=f"kT{c}") for c in range(2)]
            rkT = [pers.tile([P, L], F16, name=f"rkT{c}") for c in range(2)]
            vp = [pers.tile([P, 16, DH + 1], F16, name=f"vp{h}") for h in range(NHL)]
            ctxT = [pers.tile([P, L], F16, name=f"ctxT{c}") for c in range(2)]

            gb = lng_p.tile([P, D], F16)
            nc.gpsimd.dma_start(gb[:], gamma.ap().rearrange(
                "(o d) -> o d", o=1).to_broadcast((P, D)))
            bb = lng_p.tile([P, D], F16)
            nc.gpsimd.dma_start(bb[:], beta.ap().rearrange(
                "(o d) -> o d", o=1).to_broadcast((P, D)))

            rwb_sb = wr_p.tile([P, 2], F32)
            nc.sync.dma_start(rwb_sb[:], rwb.rearrange("(c p) -> p c", p=P))
            rrb_sb = wr_p.tile([P, 2], F32)
            nc.sync.dma_start(rrb_sb[:], rrb.rearrange("(c p) -> p c", p=P))

            wv_r = wr_p.tile([P, 8, 256], F16)
            nc.sync.dma_start(wv_r[:], Wv.rearrange("(k p) n -> p k n", p=P))

            # ---------- phase A: projections, one pass per cc ----------
            def emit_phaseA(cc):
                c0 = 128 * cc
                wq_r = wr_p.tile([P, 8, 128], F16, tag="wq", name=f"wq{cc}")
                nc.sync.dma_start(
                    wq_r[:], Wq[:, c0:c0 + 128].rearrange("(k p) n -> p k n", p=P))
                wk_r = wr_p.tile([P, 8, 128], F16, tag="wk", name=f"wk{cc}")
                nc.sync.dma_start(
                    wk_r[:], Wk[:, c0:c0 + 128].rearrange("(k p) n -> p k n", p=P))
                wl_r = wr_p.tile([P, 8, 128], F16, tag="wl", name=f"wl{cc}")
                nc.sync.dma_start(
                    wl_r[:], Wrel[:, c0:c0 + 128].rearrange("(k p) n -> p k n", p=P))

                for ic in range(8):
                    I0 = 256 * ic
                    xs = slab_p.tile([P, 8, 256], F16, tag="slab", name="xs")
                    nc.sync.dma_start(
                        xs[:], xT[:, I0:I0 + 256].rearrange("(k p) n -> p k n", p=P))
                    rsl = slab_p.tile([P, 8, 256], F16, tag="slab", name="rsl")
                    nc.sync.dma_start(
                        rsl[:], relT[:, I0:I0 + 256].rearrange("(k p) n -> p k n", p=P))

                    pq = psP.tile([P, 512], F32, tag="s", name="pq")
                    for k in range(8):
                        nc.tensor.matmul(pq[:, 0:256], wq_r[:, k, :],
                                         xs[:, k, :], start=(k == 0), stop=(k == 7))
                    nc.vector.tensor_scalar_add(rwT[cc][:, I0:I0 + 256],
                                                pq[:, 0:256], rwb_sb[:, cc:cc + 1])
                    nc.vector.tensor_scalar_add(rrT[cc][:, I0:I0 + 256],
                                                pq[:, 0:256], rrb_sb[:, cc:cc + 1])
                    pk = psP.tile([P, 512], F32, tag="s", name="pk")
                    for k in range(8):
                        nc.tensor.matmul(pk[:, 0:256], wk_r[:, k, :],
                                         xs[:, k, :], start=(k == 0), stop=(k == 7))
                    nc.scalar.copy(kT[cc][:, I0:I0 + 256], pk[:, 0:256])
                    pr = psP.tile([P, 512], F32, tag="s", name="pr")
                    for k in range(8):
                        nc.tensor.matmul(pr[:, 0:256], wl_r[:, k, :],
                                         rsl[:, k, :], start=(k == 0), stop=(k == 7))
                    nc.scalar.copy(rkT[cc][:, I0:I0 + 256], pr[:, 0:256])

                    if cc == 0:
                        for jj in range(2):
                            jo = 2 * ic + jj
                            pv = psP.tile([P, 512], F32, tag="s", name="pv")
                            for k in range(8):
                                nc.tensor.matmul(pv[:, 0:256],
                                                 xs[:, k, 128 * jj:128 * jj + 128],
                                                 wv_r[:, k, :],
                                                 start=(k == 0), stop=(k == 7))
                            for h in range(NHL):
                                nc.vector.tensor_scalar_mul(
                                    vp[h][:, jo, 0:DH], pv[:, DH * h:DH * h + DH],
                                    m01[:, jo:jo + 1])
                                nc.scalar.copy(vp[h][:, jo, DH:DH + 1],
                                               m01[:, jo:jo + 1])

            # ---------- phase B pieces (fine-grained cross-head interleave) ----
            pf_bufs = [dram.tile([PFR], F16, name=f"pf{i}") for i in range(2)]
            attn_d = dram.tile([L, D], F16)
            rs_d = dram.tile([512, D], F16)

            wo_r = [wo_p.tile([P, 2, 512], F16, name=f"wo{c}") for c in range(2)]
            for c in range(2):
                nc.sync.dma_start(
                    wo_r[c][:], Wout[128 * c:128 * c + 128, :]
                    .rearrange("p (t n) -> p t n", t=2))

            # seq of (half, h); state per head
            seq = [(hf, h) for hf in range(2) for h in range(4)]
            st = {k: dict(sh=[], p16=[], ptq=[], pc=None, ot=None, bd_i=0)
                  for k in seq}

            def head_geom(key):
                half, h = key
                return 1024 * half, h // 2, h % 2

            def bd_chunk(key, ic):
                """one bd chunk: 4 MMs + 4 drains + pf write."""
                half, h = key
                H0, cc, par = head_geom(key)
                sA = slice(64 * par, 64 * par + 64)
                pf = pf_bufs[seq.index(key) % 2][:]
                pf2d = pf[0:PFR].rearrange("(r c) -> r c", c=L + 1)
                Q0 = H0 + 128 * ic
                wt = wt_p.tile([P, L + 1], F16, tag="wt")
                nc.vector.memset(wt[:, 0:1], 0.0)
                for t in range(4):
                    pbd = psP.tile([P, 512], F32, tag="s", name="pbd")
                    nc.tensor.matmul(pbd[:], rrT[cc][sA, Q0:Q0 + 128],
                                     rkT[cc][sA, 512 * t:512 * t + 512],
                                     start=True, stop=True)
                    if t < 3:
                        nc.vector.tensor_copy(
                            wt[:, 1 + 512 * t:1 + 512 * t + 512], pbd[:])
                    else:
                        nc.scalar.copy(
                            wt[:, 1 + 512 * t:1 + 512 * t + 512], pbd[:])
                if ic < 8:
                    nc.sync.dma_start(pf2d[128 * ic:128 * ic + 128, :], wt[:])
                else:
                    nc.sync.dma_start(pf2d[1024:1025, :], wt[0:1, :])

            def sh_prefetch(key):
                half, h = key
                pf = pf_bufs[seq.index(key) % 2][:]
                off = L - 1024 * half
                for icc in range(8):
                    I0l = 128 * icc
                    sh16 = sh_p.tile([P, L], F16, tag="sh")
                    nc.sync.dma_start(
                        sh16[:],
                        pf[off + I0l * L: off + (I0l + 128) * L]
                        .rearrange("(r c) -> r c", c=L))
                    st[key]["sh"].append(sh16)

            def sc_icc(key, icc):
                """scores for one 128-row block: 8 MMs + 4 exp drains."""
                half, h = key
                H0, cc, par = head_geom(key)
                sA = slice(64 * par, 64 * par + 64)
                I0 = H0 + 128 * icc
                sh16 = st[key]["sh"][icc]
                p16 = p16_p.tile([P, L], F16, tag="p16")
                for t in range(4):
                    psc = psP.tile([P, 512], F32, tag="s", name="psc")
                    nc.tensor.matmul(psc[:], rwT[cc][sA, I0:I0 + 128],
                                     kT[cc][sA, 512 * t:512 * t + 512],
                                     start=True, stop=False)
                    nc.tensor.matmul(psc[:], ident[:],
                                     sh16[:, 512 * t:512 * t + 512],
                                     start=False, stop=True)
                    nc.scalar.activation(p16[:, 512 * t:512 * t + 512],
                                         psc[:], AF.Exp, bias=nbias[:])
                st[key]["p16"].append(p16)

            def tp_unit(key, J):
                """transpose one k-chunk of probs; ctx matmul lags one J."""
                half, h = key
                s = st[key]
                ptp = psT.tile([P, 1024], F16, tag="pt")
                for icc in range(8):
                    nc.tensor.matmul(ptp[:, 128 * icc:128 * icc + 128],
                                     s["p16"][icc][:, 128 * J:128 * J + 128],
                                     ident[:], is_transpose=True,
                                     start=True, stop=True)
                pt_sb = pt_p.tile([P, 1024], F16, tag="ptsb")
                if J % 2 == 0:
                    nc.vector.tensor_copy(pt_sb[:], ptp[:])
                else:
                    nc.scalar.copy(pt_sb[:], ptp[:])
                s["ptq"].append(pt_sb)
                if J == 1:
                    s["pc"] = (psC.tile([65, 512], F32, tag="c", name="pc0"),
                               psC.tile([65, 512], F32, tag="c", name="pc1"))
                if J >= 1:
                    Jp = J - 1
                    for ii, pc in enumerate(s["pc"]):
                        nc.tensor.matmul(pc[:], vp[h][:, Jp, :],
                                         s["ptq"][Jp][:, 512 * ii:512 * ii + 512],
                                         start=(Jp == 0), stop=False)

            def norm(key):
                """final ctx J, denominator, reciprocal, scale into ctxT."""
                half, h = key
                H0, cc, par = head_geom(key)
                s = st[key]
                for ii, pc in enumerate(s["pc"]):
                    nc.tensor.matmul(pc[:], vp[h][:, 15, :],
                                     s["ptq"][15][:, 512 * ii:512 * ii + 512],
                                     start=False, stop=True)
                cs0 = cs_p.tile([65, 512], F32, tag="cs0", name="cs0")
                cs1 = cs_p.tile([65, 512], F32, tag="cs1", name="cs1")
                nc.vector.tensor_copy(cs0[:], s["pc"][0][:])
                nc.vector.tensor_copy(cs1[:], s["pc"][1][:])
                den0 = den_p.tile([1, 1024], F32, tag="den0", name="den0")
                rec0 = den_p.tile([1, 1024], F32, tag="rec0", name="rec0")
                recr = den_p.tile([1, 1024], F16, tag="recr", name="recr")
                scr = den_p.tile([1, 1024], F32, tag="scr", name="scr")
                nc.sync.dma_start(den0[0:1, 0:512], cs0[64:65, :])
                nc.sync.dma_start(den0[0:1, 512:1024], cs1[64:65, :])
                nc.vector.reciprocal_approx_accurate(
                    rec0[0:1, :], den0[0:1, :], scr[0:1, :])
                nc.vector.tensor_copy(recr[0:1, :], rec0[0:1, :])
                if par == 1:
                    s["ot"] = odd_p.tile([64, 1024], F16, tag="odd", name="ot")
                for ii, cs in enumerate((cs0, cs1)):
                    i0 = H0 + 512 * ii
                    pb = psP.tile([P, 512], F32, tag="s", name="pb")
                    nc.tensor.matmul(pb[0:64, :], ones_r[0:1, :],
                                     recr[0:1, 512 * ii:512 * ii + 512],
                                     start=True, stop=True)
                    bc = bc_p.tile([64, 512], F32, tag="bc")
                    nc.scalar.copy(bc[:], pb[0:64, :])
                    if par == 0:
                        nc.vector.tensor_mul(ctxT[cc][0:64, i0:i0 + 512],
                                             cs[0:64, :], bc[:])
                    else:
                        nc.vector.tensor_mul(s["ot"][:, 512 * ii:512 * ii + 512],
                                             cs[0:64, :], bc[:])
                if par == 1:
                    nc.sync.dma_start(ctxT[cc][64:128, H0:H0 + 1024],
                                      s["ot"][:, :])

            def S1(tp_key, sc_key):
                """tp/ctx of previous head threaded with icc 0..3 of current."""
                for J in range(16):
                    tp_unit(tp_key, J)
                    if J in (3, 7, 11, 15):
                        sc_icc(sc_key, (J - 3) // 4)
                norm(tp_key)

            def S2(sc_key, bd_key):
                """icc 4..7 of current head threaded with bd of next head."""
                nchunk = 0
                if bd_key is not None:
                    nchunk = 9 if bd_key[0] == 0 else 8
                bi = 0
                for icc in range(4, 8):
                    for _ in range((nchunk + 3) // 4):
                        if bi < nchunk:
                            bd_chunk(bd_key, bi)
                            bi += 1
                    sc_icc(sc_key, icc)
                while bi < nchunk:
                    bd_chunk(bd_key, bi)
                    bi += 1
                if bd_key is not None:
                    sh_prefetch(bd_key)

            def proj(half):
                H0 = 1024 * half
                for icb in range(8):
                    I0 = H0 + 128 * icb
                    for t in range(2):
                        po = psP.tile([P, 512], F32, tag="s", name="po")
                        for c in range(2):
                            nc.tensor.matmul(po[:], ctxT[c][:, I0:I0 + 128],
                                             wo_r[c][:, t, :],
                                             start=(c == 0), stop=(c == 1))
                        ao = oc_p.tile([P, 512], F16, tag="ao")
                        if t == 0:
                            nc.vector.tensor_copy(ao[:], po[:])
                        else:
                            nc.scalar.copy(ao[:], po[:])
                        nc.sync.dma_start(
                            attn_d[I0:I0 + 128, 512 * t:512 * t + 512], ao[:])

            def rs(half):
                nc.gpsimd.collective_compute(
                    "ReduceScatter", OP.add,
                    replica_groups=[[0, 1, 2, 3], [4, 5, 6, 7]],
                    ins=[attn_d[1024 * half:1024 * half + 1024, :].opt()],
                    outs=[rs_d[256 * half:256 * half + 256, :].opt()],
                )

            def ln(half):
                for j in range(2):
                    R0 = 256 * half + 128 * j
                    rs16 = ln_p.tile([P, D], F16, tag="rs16", bufs=2)
                    nc.gpsimd.dma_start(rs16[:], rs_d[R0:R0 + 128, :])
                    xr = ln_p.tile([P, D], F32, tag="xr")
                    nc.gpsimd.dma_start(xr[:], xres[R0:R0 + 128, :])
                    zt = ln_p.tile([P, D], F32, tag="zt")
                    nc.vector.tensor_copy(zt[:], rs16[:])
                    nc.vector.tensor_add(zt[:], zt[:], xr[:])
                    mu = ln_p.tile([P, 1], F32, tag="mu")
                    nc.vector.tensor_reduce(mu[:], zt[:], AX.X, OP.add)
                    nc.vector.tensor_scalar_mul(mu[:], mu[:], 1.0 / D)
                    xc = ln_p.tile([P, D], F32, tag="xc")
                    nc.vector.tensor_scalar_sub(xc[:], zt[:], mu[:])
                    sq = ln_p.tile([P, D], F32, tag="zt", name="sq")
                    nc.vector.tensor_mul(sq[:], xc[:], xc[:])
                    var = ln_p.tile([P, 1], F32, tag="var")
                    nc.vector.tensor_reduce(var[:], sq[:], AX.X, OP.add)
                    nc.vector.tensor_scalar_mul(var[:], var[:], 1.0 / D)
                    nc.vector.tensor_scalar_add(var[:], var[:], LN_EPS)
                    sd = ln_p.tile([P, 1], F32, tag="sd")
                    nc.scalar.activation(sd[:], var[:], AF.Sqrt)
                    isd = ln_p.tile([P, 1], F32, tag="isd")
                    nc.vector.reciprocal(isd[:], sd[:])
                    nc.vector.tensor_scalar_mul(xc[:], xc[:], isd[:])
                    nc.vector.tensor_mul(xc[:], xc[:], gb[:])
                    nc.vector.tensor_add(xc[:], xc[:], bb[:])
                    nc.sync.dma_start(out[R0:R0 + 128, :], xc[:])

            # ---------- emission schedule ----------
            emit_phaseA(0)
            # startup: bd(0,0) alone, then phase A cc1 as filler, then
            # scores(0,0) icc0..3 without tp interleave.
            for bi in range(9):
                bd_chunk((0, 0), bi)
            sh_prefetch((0, 0))
            emit_phaseA(1)
            for icc in range(4):
                sc_icc((0, 0), icc)
            # steady-state pipeline
            for k in range(len(seq)):
                bd_key = seq[k + 1] if k + 1 < len(seq) else None
                S2(seq[k], bd_key)
                if k + 1 < len(seq):
                    S1(seq[k], seq[k + 1])
                else:
                    for J in range(16):
                        tp_unit(seq[k], J)
                    norm(seq[k])
                # tail work threaded between sections
                if seq[k] == (0, 3):
                    proj(0)          # after norm(0,3): half-0 ctxT complete
                if seq[k] == (1, 0):
                    rs(0)
                if seq[k] == (1, 2):
                    ln(0)
            proj(1)
            rs(1)
            ln(1)

    nc.compile()
    return nc


def _prep_inputs(x, relative_pos, r_w_bias, r_r_bias, attn_mask,
                 W_qkv, W_rel, W_out, ln_gamma, ln_beta):
    in_maps = []
    relT = np.ascontiguousarray(relative_pos.T).astype(np.float16)
    m01f = (~np.asarray(attn_mask).astype(bool)).astype(np.float32)
    for c in range(N_CORES):
        b, g = c // 4, c % 4
        h0 = 4 * g
        cols = slice(DH * h0, DH * h0 + 256)
        im = dict(
            xT=np.ascontiguousarray(x[b].T).astype(np.float16),
            relT=relT,
            xres=np.ascontiguousarray(np.concatenate(
                [x[b, 256 * g:256 * g + 256, :],
                 x[b, 1024 + 256 * g:1024 + 256 * g + 256, :]],
                axis=0)).astype(np.float32),
            Wq=np.ascontiguousarray(
                W_qkv[:, DH * h0:DH * h0 + 256] * SCALE).astype(np.float16),
            Wk=np.ascontiguousarray(
                W_qkv[:, D + DH * h0: D + DH * h0 + 256]).astype(np.float16),
            Wv=np.ascontiguousarray(
                W_qkv[:, 2 * D + DH * h0: 2 * D + DH * h0 + 256]).astype(np.float16),
            Wrel=np.ascontiguousarray(W_rel[:, cols]).astype(np.float16),
            Wout=np.ascontiguousarray(W_out[cols, :]).astype(np.float16),
            rwb=np.ascontiguousarray(
                r_w_bias[h0:h0 + 4].reshape(-1) * SCALE).astype(np.float32),
            rrb=np.ascontiguousarray(
                r_r_bias[h0:h0 + 4].reshape(-1) * SCALE).astype(np.float32),
            mask01=m01f[b],
            gamma=np.asarray(ln_gamma).astype(np.float16),
            beta=np.asarray(ln_beta).astype(np.float16),
        )
        in_maps.append(im)
    return in_maps


def kernel(**inputs):
    from concourse.bass_utils import run_bass_kernel_spmd

    if "nc" not in _CACHE:
        _CACHE["nc"] = _build_program()
    nc = _CACHE["nc"]

    in_maps = _prep_inputs(**{k: np.asarray(v) for k, v in inputs.items()})
    res = run_bass_kernel_spmd(nc, in_maps, list(range(N_CORES)))
    outp = np.empty((B, L, D), np.float32)
    for c in range(N_CORES):
        b, g = c // 4, c % 4
        o = res.results[c]["out"]
        outp[b, 256 * g:256 * g + 256, :] = o[0:256, :]
        outp[b, 1024 + 256 * g:1024 + 256 * g + 256, :] = o[256:512, :]
    return outp

